# revision 4
# baseline (speedup 1.0000x reference)
"""BilinearAttention Trainium2 kernel (8 NeuronCores, SPMD, no collectives).

Problem (hardcoded): B=4, C=256, H=W=64 (HW=4096)
  theta = convbn_theta(x)   -> [B, 32, HW]
  phi   = convbn_phi(fea)   -> [B, 32, HW]
  g     = convbn_g(fea)     -> [B, 128, HW]
  attn  = softmax_m(theta^T . phi)          [B, HW(n), HW(m)]
  out   = g @ attn^T                         [B, 128, HW]
  final = convbn_fin(out)                    [B, 256, H, W]

Sharding: core k handles (b = k//2, n-half h = k%2): 2048 queries x 4096 keys.

Device algorithm per core (all matmuls float32r ~ tf32, psum fp32):
  - Fold BN into conv weights on host. theta/phi biases kept; g bias folded
    into fin bias (attn rows sum to 1); fin bias applied at the tail.
  - theta_rep [128,2048]: 4 partition-group copies of theta[32, n] (for 4-way
    row-tiled K=32 QK matmuls). Same for phi_rep [128, 4096].
  - gT [128, 4096]: column block mt holds g^T for m-tile mt ([128 m, 128 c]).
  - Main loop over 128 tasks (nb-major; task = (m-tile mt, n-block nb)):
      QK:  logitsT[m,n] psum = phi_mt^T theta_nb   (3 tasks row-packed / psum quad)
      exp: ACT psum->sbuf f32r (one [128, 3*512] instr per triple; no max
           subtraction needed: |logit| < ~40 so exp fits fp32 comfortably)
      AV:  av_ps[nb]  += gT_mt^T . pT   (psum accumulate over the 32 m-tiles)
      s:   s_ps[nb]   += ones^T . pT    (softmax denominators)
  - Tail: r = 1/s; broadcast via (1/32)-matmul; avn = av * r; fin conv; + bias.
"""
import numpy as np
from contextlib import ExitStack

B, C, HW = 4, 256, 4096
NSH = HW // 2           # 2048 queries per core
NCORES = 8
BN_EPS = 1e-5
NB = 4                  # n-blocks of 512 per core
MT = 32                 # m-tiles of 128
TRIPLE = 3              # QK tasks per exp instruction

_CACHE = {}


def _build():
    if "nc" in _CACHE:
        return _CACHE["nc"]
    import concourse.bacc as bacc
    import concourse.tile as tile
    from concourse import mybir

    F32 = mybir.dt.float32
    F32R = mybir.dt.float32r
    AF = mybir.ActivationFunctionType

    nc = bacc.Bacc("TRN2", target_bir_lowering=False, debug=False,
                   num_devices=NCORES)

    xk_d = nc.dram_tensor("xk", [C, NSH], F32R, kind="ExternalInput").ap()
    fea_d = nc.dram_tensor("fea", [C, HW], F32R, kind="ExternalInput").ap()
    thw_d = nc.dram_tensor("thw", [C, 128], F32R, kind="ExternalInput").ap()
    phw_d = nc.dram_tensor("phw", [C, 128], F32R, kind="ExternalInput").ap()
    gwt_d = nc.dram_tensor("gwt", [C, 128], F32R, kind="ExternalInput").ap()
    fwt_d = nc.dram_tensor("fwt", [128, 256], F32R, kind="ExternalInput").ap()
    ones_d = nc.dram_tensor("ones", [128, 128], F32R, kind="ExternalInput").ap()
    inv32_d = nc.dram_tensor("inv32", [32, 128], F32R, kind="ExternalInput").ap()
    tth_d = nc.dram_tensor("tth", [128, 1], F32, kind="ExternalInput").ap()
    tph_d = nc.dram_tensor("tph", [128, 1], F32, kind="ExternalInput").ap()
    tfn_d = nc.dram_tensor("tfn", [128, 2], F32, kind="ExternalInput").ap()
    out_d = nc.dram_tensor("out", [256, NSH], F32, kind="ExternalOutput").ap()

    with tile.TileContext(nc) as tc, ExitStack() as ctx:
        consts = ctx.enter_context(tc.tile_pool(name="consts", bufs=1))
        inbufs = ctx.enter_context(tc.tile_pool(name="inbufs", bufs=4))
        big = ctx.enter_context(tc.tile_pool(name="big", bufs=1))
        ptp = ctx.enter_context(tc.tile_pool(name="ptp", bufs=2))
        psum = ctx.enter_context(tc.tile_pool(name="psum", bufs=2, space="PSUM"))

        def dmac(name, shape, dt, src):
            t = consts.tile(shape, dt, name=name, tag=name)
            nc.sync.dma_start(out=t, in_=src)
            return t

        thw = [dmac(f"thw{i}", [128, 128], F32R, thw_d[128 * i:128 * (i + 1), :]) for i in range(2)]
        phw = [dmac(f"phw{i}", [128, 128], F32R, phw_d[128 * i:128 * (i + 1), :]) for i in range(2)]
        gwt = [dmac(f"gwt{i}", [128, 128], F32R, gwt_d[128 * i:128 * (i + 1), :]) for i in range(2)]
        fwt = dmac("fwt_t", [128, 256], F32R, fwt_d)
        ones = dmac("ones_t", [128, 128], F32R, ones_d)
        inv32 = dmac("inv32_t", [32, 128], F32R, inv32_d)
        tth = dmac("tth_t", [128, 1], F32, tth_d)
        tph = dmac("tph_t", [128, 1], F32, tph_d)
        tfn = dmac("tfn_t", [128, 2], F32, tfn_d)

        theta_rep = big.tile([128, NSH], F32R)
        phi_rep = big.tile([128, HW], F32R)
        gT = big.tile([128, HW], F32R)
        av_sb = big.tile([128, NSH], F32)
        avn = big.tile([128, NSH], F32R)
        s_sb = big.tile([32, NSH], F32R)
        r_sb = big.tile([32, NSH], F32R)
        out_sb = big.tile([128, 2 * NSH], F32)

        # ---- prologue: theta conv over the core's n-range ----
        for j in range(4):
            xk0 = inbufs.tile([128, 512], F32R, tag="xk0")
            xk1 = inbufs.tile([128, 512], F32R, tag="xk1")
            nc.sync.dma_start(out=xk0, in_=xk_d[0:128, 512 * j:512 * (j + 1)])
            nc.sync.dma_start(out=xk1, in_=xk_d[128:256, 512 * j:512 * (j + 1)])
            ps = psum.tile([128, 512], F32, tag="qk")
            nc.tensor.matmul(ps, lhsT=thw[0], rhs=xk0, start=True, stop=False)
            nc.tensor.matmul(ps, lhsT=thw[1], rhs=xk1, start=False, stop=True)
            nc.vector.tensor_scalar_add(theta_rep[:, 512 * j:512 * (j + 1)], ps, tth)

        # ---- prologue: phi conv + gT conv per 512-wide m-chunk ----
        for j in range(8):
            f0 = inbufs.tile([128, 512], F32R, tag="f0")
            f1 = inbufs.tile([128, 512], F32R, tag="f1")
            nc.sync.dma_start(out=f0, in_=fea_d[0:128, 512 * j:512 * (j + 1)])
            nc.sync.dma_start(out=f1, in_=fea_d[128:256, 512 * j:512 * (j + 1)])
            ps = psum.tile([128, 512], F32, tag="qk")
            nc.tensor.matmul(ps, lhsT=phw[0], rhs=f0, start=True, stop=False)
            nc.tensor.matmul(ps, lhsT=phw[1], rhs=f1, start=False, stop=True)
            nc.vector.tensor_scalar_add(phi_rep[:, 512 * j:512 * (j + 1)], ps, tph)
            ps2 = psum.tile([128, 512], F32, tag="qk")
            for t in range(4):
                sl = slice(128 * t, 128 * (t + 1))
                nc.tensor.matmul(ps2[:, sl], lhsT=f0[:, sl], rhs=gwt[0],
                                 start=True, stop=False)
                nc.tensor.matmul(ps2[:, sl], lhsT=f1[:, sl], rhs=gwt[1],
                                 start=False, stop=True)
            nc.vector.tensor_copy(gT[:, 512 * j:512 * (j + 1)], ps2)

        # ---- main loop ----
        tasks = [(i % MT, i // MT) for i in range(MT * NB)]  # (mt, nb), nb-major
        triples = [tasks[i:i + TRIPLE] for i in range(0, len(tasks), TRIPLE)]
        nt = len(triples)
        av_ps = [None] * NB
        s_ps = [None] * NB
        quads = [None] * nt
        pts = [None] * nt

        def emit_qk(i):
            grp = triples[i]
            q = psum.tile([128, 512 * len(grp)], F32, tag="qk")
            quads[i] = q
            for jj, (mt, nb) in enumerate(grp):
                r = mt % 4
                nc.tensor.matmul(
                    q[:, 512 * jj:512 * (jj + 1)],
                    lhsT=phi_rep[32 * r:32 * (r + 1), 128 * mt:128 * (mt + 1)],
                    rhs=theta_rep[32 * r:32 * (r + 1), 512 * nb:512 * (nb + 1)],
                    start=True, stop=True, tile_position=(32 * r, 0),
                )

        def emit_exp(i):
            q = quads[i]
            pt = ptp.tile([128, q.shape[-1]], F32R, tag="pt")
            pts[i] = pt
            nc.scalar.activation(out=pt, in_=q, func=AF.Exp)

        def emit_avs(i):
            grp = triples[i]
            pt = pts[i]
            for jj, (mt, nb) in enumerate(grp):
                if mt == 0:
                    av_ps[nb] = psum.tile([128, 512], F32, tag="av", bufs=1, name=f"av_ps{nb}")
                    s_ps[nb] = psum.tile([128, 512], F32, tag="sp", bufs=1, name=f"s_ps{nb}")
                sl = slice(512 * jj, 512 * (jj + 1))
                nc.tensor.matmul(av_ps[nb], lhsT=gT[:, 128 * mt:128 * (mt + 1)],
                                 rhs=pt[:, sl], start=(mt == 0), stop=(mt == MT - 1),
                                 skip_group_check=True)
                nc.tensor.matmul(s_ps[nb], lhsT=ones, rhs=pt[:, sl],
                                 start=(mt == 0), stop=(mt == MT - 1),
                                 skip_group_check=True)
                if mt == MT - 1:
                    nc.vector.tensor_copy(av_sb[:, 512 * nb:512 * (nb + 1)], av_ps[nb])
                    nc.vector.tensor_copy(s_sb[:, 512 * nb:512 * (nb + 1)],
                                          s_ps[nb][0:32, :])

        emit_qk(0)
        for i in range(nt):
            emit_exp(i)
            if i + 1 < nt:
                emit_qk(i + 1)
            emit_avs(i)

        # ---- tail: normalize, fin conv, bias, store ----
        with nc.allow_low_precision(reason="f32r softmax normalization"):
            nc.vector.reciprocal(r_sb, s_sb)
            for nb in range(NB):
                sl = slice(512 * nb, 512 * (nb + 1))
                rb = psum.tile([128, 512], F32, tag="av", bufs=1)
                nc.tensor.matmul(rb, lhsT=inv32, rhs=r_sb[:, sl], start=True, stop=True)
                nc.vector.tensor_tensor(avn[:, sl], av_sb[:, sl], rb,
                                        mybir.AluOpType.mult)
            for oh in range(2):
                for nb in range(NB):
                    sl = slice(512 * nb, 512 * (nb + 1))
                    fp = psum.tile([128, 512], F32, tag="sp", bufs=1)
                    nc.tensor.matmul(fp, lhsT=fwt[:, 128 * oh:128 * (oh + 1)],
                                     rhs=avn[:, sl], start=True, stop=True)
                    osl = slice(NSH * oh + 512 * nb, NSH * oh + 512 * (nb + 1))
                    nc.vector.tensor_scalar_add(out_sb[:, osl], fp, tfn[:, oh:oh + 1])
                    nc.sync.dma_start(out=out_d[128 * oh:128 * (oh + 1), sl],
                                      in_=out_sb[:, osl])

    nc.compile()
    _CACHE["nc"] = nc
    return nc


def _fold_bn(w, b, gamma, beta, mean, var):
    s = gamma / np.sqrt(var + BN_EPS)
    return (w * s[:, None]).astype(np.float32), ((b - mean) * s + beta).astype(np.float32)


def kernel(x, fea,
           theta_w, theta_b, theta_gamma, theta_beta, theta_mean, theta_var,
           phi_w, phi_b, phi_gamma, phi_beta, phi_mean, phi_var,
           g_w, g_b, g_gamma, g_beta, g_mean, g_var,
           fin_w, fin_b, fin_gamma, fin_beta, fin_mean, fin_var,
           _trace=False, _trace_kwargs=None):
    from concourse.bass_utils import run_bass_kernel_spmd

    nc = _build()

    x = np.asarray(x, np.float32)
    fea = np.asarray(fea, np.float32)
    thw_eff, t_th = _fold_bn(np.asarray(theta_w, np.float32), theta_b, theta_gamma,
                             theta_beta, theta_mean, theta_var)
    phw_eff, t_ph = _fold_bn(np.asarray(phi_w, np.float32), phi_b, phi_gamma,
                             phi_beta, phi_mean, phi_var)
    gw_eff, t_g = _fold_bn(np.asarray(g_w, np.float32), g_b, g_gamma,
                           g_beta, g_mean, g_var)
    fw_eff, t_fn = _fold_bn(np.asarray(fin_w, np.float32), fin_b, fin_gamma,
                            fin_beta, fin_mean, fin_var)
    t_fn_adj = (fw_eff @ t_g + t_fn).astype(np.float32)

    common = {
        "thw": np.ascontiguousarray(np.tile(thw_eff.T, (1, 4))),
        "phw": np.ascontiguousarray(np.tile(phw_eff.T, (1, 4))),
        "gwt": np.ascontiguousarray(gw_eff.T),
        "fwt": np.ascontiguousarray(fw_eff.T),
        "ones": np.ones((128, 128), np.float32),
        "inv32": np.full((32, 128), 1.0 / 32.0, np.float32),
        "tth": np.ascontiguousarray(np.tile(t_th, 4)[:, None]),
        "tph": np.ascontiguousarray(np.tile(t_ph, 4)[:, None]),
        "tfn": np.ascontiguousarray(t_fn_adj.reshape(2, 128).T),
    }
    xf = x.reshape(B, C, HW)
    ff = fea.reshape(B, C, HW)
    in_maps = []
    for k in range(NCORES):
        b, h = k // 2, k % 2
        m = dict(common)
        m["xk"] = np.ascontiguousarray(xf[b, :, NSH * h:NSH * (h + 1)])
        m["fea"] = np.ascontiguousarray(ff[b])
        in_maps.append(m)

    kw = {}
    if _trace:
        kw["trace"] = True
        kw.update(_trace_kwargs or {})
    res = run_bass_kernel_spmd(nc, in_maps, list(range(NCORES)), **kw)

    out = np.empty((B, C, HW), np.float32)
    for k in range(NCORES):
        b, h = k // 2, k % 2
        out[b, :, NSH * h:NSH * (h + 1)] = res.results[k]["out"]
    out = out.reshape(B, C, 64, 64)
    if _trace:
        kernel.last_results = res
    return out
